# revision 46
# baseline (speedup 1.0000x reference)
"""DSNT + JSD + distance double loss on 8 TRN2 NeuronCores.

Data-parallel: batch 64 is split into 8 shards of 8 batches; each core
computes its partial sum s_i over its 16 (b,c) heatmap slices; the host
sums the 8 partials and divides by B.

Per (b,c) slice (512x512 -> SBUF [128, 2048], partition p holds rows
h in {4p..4p+3}):
  e    = exp(x)                 ACT, accum -> per-(p,j) rowsums
  cols = ones^T @ e             PE -> PSUM [1,512] col sums
  S    = sum(e); invS broadcast via PE
  m2   = e*invS + t (= p + t)   DVE fused scalar_tensor_tensor, accum -> sum(m2)
  l    = ln(m2)                 ACT
  sum(m2^2)                     ACT square with accum
  sum(m2*l)                     GpSimd stt with accum
  argmax(t): DVE max8 + max_index (exact first-occurrence), combined
  across partitions with gpsimd partition_all_reduce (max) and a
  masked-min on the flat index.
jsd total = [0.5*sum(m2*l) - 0.5*ln2*sum(m2) - 0.25*sum(m2^2)] / (H*W)
"""

import math
import os

import numpy as np

import concourse.bacc as bacc
import concourse.bass as bass
import concourse.mybir as mybir
import concourse.tile as tile
from concourse.bass_utils import run_bass_kernel_spmd

F32 = mybir.dt.float32
BF16 = mybir.dt.bfloat16
U32 = mybir.dt.uint32
I32 = mybir.dt.int32
ALU = mybir.AluOpType
ACTF = mybir.ActivationFunctionType
AX = mybir.AxisListType

B, C, H, W = 64, 2, 512, 512
N_CORES = 8
B_SH = B // N_CORES          # 8 batches per core
NSL = B_SH * C               # 16 slices per core
P = 128                      # SBUF partitions
FD = (H * W) // P            # 2048 free elements per partition
SUB = W                      # 512-wide sub-columns (4 per row)
NSUB = FD // SUB             # 4

# Where the big sum(m2*l) product runs: "gpsimd" frees the DVE for the
# argmax passes (GpSimd only contends with DVE 2-src ops for the shared
# SBUF port); "vector" is the safe fallback.
M2L_ENGINE = os.environ.get("K_M2L_ENGINE", "gpsimd")
SQ_ENGINE = os.environ.get("K_SQ_ENGINE", "scalar")


_CACHE = {}
LAST_RESULTS = None


def _constants():
    # partition p, sub-row j holds image row h = 4p + j
    hidx = (np.arange(P, dtype=np.float32)[:, None] * NSUB
            + np.arange(NSUB, dtype=np.float32)[None, :])
    ys = (hidx + 1.0) / H                                   # [128, 4]
    # lhsT for the fused col-sum matmuls: col 2j = ones, col 2j+1 = ys[:, j]
    oy = np.zeros((P, 2 * NSUB), dtype=np.float32)
    for j in range(NSUB):
        oy[:, 2 * j] = 1.0
        oy[:, 2 * j + 1] = ys[:, j]
    # weights for the [2, 512] px/py dot: row0 = xs, row1 = ones
    xs = (np.arange(W, dtype=np.float32) + 1.0) / W
    xo = np.stack([xs, np.ones(W, dtype=np.float32)])       # [2, 512]
    pbase = (np.arange(P, dtype=np.float32) * FD).reshape(P, 1)
    ones = np.ones((P, 1), dtype=np.float32)
    onesr = np.ones((1, P), dtype=np.float32)
    eye = np.eye(P, dtype=np.float32)
    import ml_dtypes
    onesb = np.ones((P, 1), dtype=ml_dtypes.bfloat16)
    return {"oy_c": oy, "xo_c": xo, "pbase_c": pbase,
            "ones_c": ones, "onesr_c": onesr, "onesb_c": onesb,
            "eye_c": eye}


def _patch_act_tables():
    """Steer the act-table chooser so Exp/Ln/Square/Copy all live in the
    single `natural_log_exp_and_others` set — otherwise the per-slice
    Exp->Ln alternation reloads tables (~1.3us each, 32x per core).
    Set ids stay aligned with act_info.json (same list, same order; only
    membership of the non-preferred sets is pruned)."""
    if _CACHE.get("act_patched"):
        return
    import concourse.hw_specs as hw_specs

    orig = hw_specs.get_activation_tables
    hot = {ACTF.Exp, ACTF.Ln, ACTF.Square, ACTF.Copy, ACTF.Identity}

    def patched(module_arch):
        tabs = orig(module_arch)
        out = {}
        for name, funcs in tabs.items():
            if name == "natural_log_exp_and_others":
                out[name] = set(funcs)
            else:
                out[name] = set(funcs) - hot
        return out

    hw_specs.get_activation_tables = patched
    bacc.get_activation_tables = patched
    _CACHE["act_patched"] = True


def build_program():
    """Build (once) the single-core Bass/Tile program run SPMD on 8 cores."""
    if "nc" in _CACHE:
        return _CACHE["nc"]

    _patch_act_tables()
    nc = bacc.Bacc("TRN2", target_bir_lowering=False, debug=False,
                   num_devices=N_CORES)

    x_d = nc.dram_tensor("x", [NSL, P, FD], F32, kind="ExternalInput").ap()
    t_d = nc.dram_tensor("t", [NSL, P, FD], F32, kind="ExternalInput").ap()
    xs_d = nc.dram_tensor("xo_c", [2, W], F32, kind="ExternalInput").ap()
    ys_d = nc.dram_tensor("oy_c", [P, 2 * NSUB], F32,
                          kind="ExternalInput").ap()
    pb_d = nc.dram_tensor("pbase_c", [P, 1], F32, kind="ExternalInput").ap()
    on_d = nc.dram_tensor("ones_c", [P, 1], F32, kind="ExternalInput").ap()
    onr_d = nc.dram_tensor("onesr_c", [1, P], F32, kind="ExternalInput").ap()
    onb_d = nc.dram_tensor("onesb_c", [P, 1], BF16, kind="ExternalInput").ap()
    eye_d = nc.dram_tensor("eye_c", [P, P], F32, kind="ExternalInput").ap()
    out_d = nc.dram_tensor("out", [1, 1], F32, kind="ExternalOutput").ap()

    with tile.TileContext(nc) as tc:
        _emit(nc, tc, x_d, t_d, xs_d, ys_d, pb_d, on_d, onr_d, onb_d, eye_d,
              out_d)

    nc.compile()
    _CACHE["nc"] = nc
    return nc


def _emit(nc, tc, x_d, t_d, xs_d, ys_d, pb_d, on_d, onr_d, onb_d, eye_d,
          out_d):
    from contextlib import ExitStack
    ctx = ExitStack()
    with ctx:
        singles = ctx.enter_context(tc.tile_pool(name="singles", bufs=1))
        xp = ctx.enter_context(tc.tile_pool(name="xp", bufs=3))
        tp = ctx.enter_context(tc.tile_pool(name="tp", bufs=3))
        ep = ctx.enter_context(tc.tile_pool(name="ep", bufs=3))
        m2p = ctx.enter_context(tc.tile_pool(name="m2p", bufs=3))
        lp = ctx.enter_context(tc.tile_pool(name="lp", bufs=3))
        scr = ctx.enter_context(tc.tile_pool(name="scr", bufs=3))
        sm = ctx.enter_context(tc.tile_pool(name="sm", bufs=4))
        pcols = ctx.enter_context(
            tc.tile_pool(name="pcols", bufs=2, space="PSUM"))
        pwsum = ctx.enter_context(
            tc.tile_pool(name="pwsum", bufs=1, space="PSUM"))
        ps_s = ctx.enter_context(
            tc.tile_pool(name="ps_s", bufs=1, space="PSUM"))
        ps_inv = ctx.enter_context(
            tc.tile_pool(name="ps_inv", bufs=2, space="PSUM"))
        pbig = ctx.enter_context(
            tc.tile_pool(name="pbig", bufs=1, space="PSUM"))

        # ---- constants (DVE HWDGE queue: the SP queue is saturated by
        # the 32 big input loads and would delay these ~18us) ----
        xo_sb = singles.tile([2, W], F32)
        nc.scalar.dma_start(out=xo_sb, in_=xs_d)
        oy_sb = singles.tile([P, 2 * NSUB], F32)
        nc.scalar.dma_start(out=oy_sb, in_=ys_d)
        pb_sb = singles.tile([P, 1], F32)
        nc.scalar.dma_start(out=pb_sb, in_=pb_d)
        ones_sb = singles.tile([P, 1], F32)
        nc.scalar.dma_start(out=ones_sb, in_=on_d)
        onesr_sb = singles.tile([1, P], F32)
        nc.scalar.dma_start(out=onesr_sb, in_=onr_d)
        onesb_sb = singles.tile([P, 1], BF16)
        nc.scalar.dma_start(out=onesb_sb, in_=onb_d)
        eye_sb = singles.tile([P, P], F32)
        nc.scalar.dma_start(out=eye_sb, in_=eye_d)

        # ---- accumulators across slices ----
        # stats columns: [0:16]=sum m2, [16:32]=sum m2^2
        stats = singles.tile([P, 2 * NSL], F32)
        wtot_all = singles.tile([1, NSL], F32)
        pmax_all = singles.tile([P, NSL], F32)
        flat_all = singles.tile([P, NSL], F32)
        pxpy_all = singles.tile([2, NSL], F32)
        invS_all = singles.tile([1, NSL], F32)

        for s in range(NSL):
            # ---- loads ----
            x_sb = xp.tile([P, FD], F32, tag="x")
            nc.sync.dma_start(out=x_sb, in_=x_d[s])
            t_sb = tp.tile([P, FD], F32, tag="t")
            nc.scalar.dma_start(out=t_sb, in_=t_d[s])

            # ---- softmax stats ----
            e_sb = ep.tile([P, FD], F32, tag="e")
            rowe = sm.tile([P, 1], F32, tag="rowe")
            nc.scalar.activation(out=e_sb, in_=x_sb, func=ACTF.Exp,
                                 accum_out=rowe)
            # fused col sums: row0 = sum_h e, row1 = sum_h ys[h]*e
            cols2 = pcols.tile([2, W], F32, tag="cols")
            for j in range(NSUB):
                nc.tensor.matmul(
                    cols2[0:2, :], lhsT=oy_sb[:, 2 * j:2 * j + 2],
                    rhs=e_sb[:, j * SUB:(j + 1) * SUB],
                    start=(j == 0), stop=(j == NSUB - 1))

            # S and 1/S (broadcast to all partitions through PE)
            s_ps = ps_s.tile([1, 1], F32, tag="s_ps")
            nc.tensor.matmul(s_ps[0:1, 0:1], lhsT=ones_sb[:, 0:1],
                             rhs=rowe[:, 0:1], start=True, stop=True)
            nc.vector.reciprocal(out=invS_all[0:1, s:s + 1],
                                 in_=s_ps[0:1, 0:1])
            invs_ps = ps_inv.tile([P, 1], F32, tag="invs_ps")
            nc.tensor.matmul(invs_ps[:, 0:1], lhsT=onesr_sb[0:1, :],
                             rhs=invS_all[0:1, s:s + 1],
                             start=True, stop=True)
            invs_sb = sm.tile([P, 1], F32, tag="invs_sb")
            nc.vector.tensor_copy(out=invs_sb, in_=invs_ps[:, 0:1])

            # px_u (row 0) and py_u (row 1) in one fused [2, 512] dot
            pxscr = sm.tile([2, W], F32, tag="pxscr")
            nc.vector.scalar_tensor_tensor(
                out=pxscr, in0=cols2[0:2, :], scalar=1.0, in1=xo_sb,
                op0=ALU.mult, op1=ALU.mult,
                accum_out=pxpy_all[0:2, s:s + 1])

            # ---- m2 = e/S + t, sum(m2); bf16 m2/l halve the shared-port
            # traffic of the Pool product (accum stays f32) ----
            m2_sb = m2p.tile([P, FD], BF16, tag="m2")
            nc.vector.scalar_tensor_tensor(
                out=m2_sb, in0=e_sb, scalar=invs_sb[:, 0:1], in1=t_sb,
                op0=ALU.mult, op1=ALU.add,
                accum_out=stats[:, s:s + 1])

            # ---- l = ln(m2); sum(m2*l); sum(m2^2) ----
            l_sb = lp.tile([P, FD], BF16, tag="l")
            nc.scalar.activation(out=l_sb, in_=m2_sb, func=ACTF.Ln)

            # w = m2 * l (Pool engine), summed via PE column-sums then a
            # tiny ACT copy-accum for the per-slice total
            w_sb = scr.tile([P, FD], BF16, tag="w")
            eng = nc.gpsimd if M2L_ENGINE == "gpsimd" else nc.vector
            eng.tensor_tensor(out=w_sb, in0=m2_sb, in1=l_sb, op=ALU.mult)
            wsum = pwsum.tile([1, W], F32, tag="wsum")
            for j in range(NSUB):
                nc.tensor.matmul(
                    wsum[0:1, :], lhsT=onesb_sb[:, 0:1],
                    rhs=w_sb[:, j * SUB:(j + 1) * SUB],
                    start=(j == 0), stop=(j == NSUB - 1))
            wscr = sm.tile([1, W], F32, tag="wscr")
            nc.scalar.activation(out=wscr, in_=wsum[0:1, :], func=ACTF.Copy,
                                 accum_out=wtot_all[0:1, s:s + 1])

            sq_sb = scr.tile([P, FD], BF16, tag="sq")
            nc.scalar.activation(
                out=sq_sb, in_=m2_sb, func=ACTF.Square,
                accum_out=stats[:, NSL + s:NSL + s + 1])

            # ---- argmax of target (exact, first occurrence) ----
            mx8 = sm.tile([P, 8], F32, tag="mx8")
            ix8 = sm.tile([P, 8], U32, tag="ix8")
            nc.vector.max(out=mx8, in_=t_sb)
            nc.vector.max_index(out=ix8, in_max=mx8, in_values=t_sb)
            nc.vector.tensor_copy(out=pmax_all[:, s:s + 1], in_=mx8[:, 0:1])
            pidxf = sm.tile([P, 1], F32, tag="pidxf")
            nc.vector.tensor_copy(out=pidxf, in_=ix8[:, 0:1])
            nc.vector.tensor_tensor(out=flat_all[:, s:s + 1],
                                    in0=pidxf, in1=pb_sb, op=ALU.add)

        # ================= end-of-loop combine =================
        fin = singles

        # cross-partition sums of all per-partition stats in one matmul
        sums_ps = pbig.tile([1, 2 * NSL], F32)
        nc.tensor.matmul(sums_ps[0:1, :], lhsT=ones_sb[:, 0:1],
                         rhs=stats, start=True, stop=True)

        # py_u lives on partition 1 of pxpy_all; hop it to partition 0
        pyu_row = fin.tile([1, NSL], F32)
        nc.sync.dma_start(out=pyu_row, in_=pxpy_all[1:2, :])

        # per-slice global max via PE transpose (PE is idle in the tail,
        # unlike the Pool engine which is still finishing products)
        pmaxT = pcols.tile([NSL, P], F32, tag="cols")
        nc.tensor.transpose(pmaxT[0:NSL, :], pmax_all, eye_sb)
        m_col = fin.tile([NSL, 1], F32)
        nc.vector.reduce_max(out=m_col, in_=pmaxT[0:NSL, :], axis=AX.X)
        m_row = pcols.tile([1, NSL], F32, tag="cols")
        nc.tensor.transpose(m_row[0:1, :], m_col, eye_sb[0:NSL, 0:NSL])
        m_row_sb = fin.tile([1, NSL], F32)
        nc.vector.tensor_copy(out=m_row_sb, in_=m_row[0:1, :])
        m_rep = pcols.tile([P, NSL], F32, tag="cols")
        nc.tensor.matmul(m_rep[:, :], lhsT=onesr_sb[0:1, :],
                         rhs=m_row_sb, start=True, stop=True)
        mk = fin.tile([P, NSL], F32)
        nc.vector.tensor_tensor(out=mk, in0=pmax_all, in1=m_rep[:, :],
                                op=ALU.is_lt)
        # first occurrence = min(flat + 1e9*mk); realized as
        # max(-(flat + 1e9*mk)) = max((mk * -1e9) - flat)
        fneg = fin.tile([P, NSL], F32)
        nc.vector.scalar_tensor_tensor(
            out=fneg, in0=mk, scalar=-1.0e9, in1=flat_all,
            op0=ALU.mult, op1=ALU.subtract)
        fnegT = pcols.tile([NSL, P], F32, tag="cols")
        nc.tensor.transpose(fnegT[0:NSL, :], fneg, eye_sb)
        fmax_col = fin.tile([NSL, 1], F32)
        nc.vector.reduce_max(out=fmax_col, in_=fnegT[0:NSL, :], axis=AX.X)
        fmin_col = fin.tile([NSL, 1], F32)
        nc.vector.tensor_scalar(out=fmin_col, in0=fmax_col, scalar1=-1.0,
                                scalar2=None, op0=ALU.mult)
        f_row = pcols.tile([1, NSL], F32, tag="cols")
        nc.tensor.transpose(f_row[0:1, :], fmin_col, eye_sb[0:NSL, 0:NSL])
        F_sb = fin.tile([1, NSL], F32)
        nc.vector.tensor_copy(out=F_sb, in_=f_row[0:1, :])

        # decompose flat -> (h, w); tx = (w+1)/W, ty = (h+1)/H
        Fi = fin.tile([1, NSL], I32)
        nc.vector.tensor_copy(out=Fi, in_=F_sb)
        wi = fin.tile([1, NSL], I32)
        nc.vector.tensor_scalar(out=wi, in0=Fi, scalar1=W - 1,
                                scalar2=None, op0=ALU.bitwise_and)
        hi = fin.tile([1, NSL], I32)
        nc.vector.tensor_scalar(out=hi, in0=Fi, scalar1=9,
                                scalar2=None, op0=ALU.arith_shift_right)
        wf = fin.tile([1, NSL], F32)
        nc.vector.tensor_copy(out=wf, in_=wi)
        hf = fin.tile([1, NSL], F32)
        nc.vector.tensor_copy(out=hf, in_=hi)
        tx = fin.tile([1, NSL], F32)
        nc.vector.tensor_scalar(out=tx, in0=wf, scalar1=1.0,
                                scalar2=1.0 / W, op0=ALU.add, op1=ALU.mult)
        ty = fin.tile([1, NSL], F32)
        nc.vector.tensor_scalar(out=ty, in0=hf, scalar1=1.0,
                                scalar2=1.0 / H, op0=ALU.add, op1=ALU.mult)

        # px, py
        px = fin.tile([1, NSL], F32)
        nc.vector.tensor_tensor(out=px, in0=pxpy_all[0:1, :], in1=invS_all,
                                op=ALU.mult)
        py = fin.tile([1, NSL], F32)
        nc.vector.tensor_tensor(out=py, in0=pyu_row, in1=invS_all,
                                op=ALU.mult)

        # ed = sqrt((tx-px)^2 + (ty-py)^2), summed
        dx = fin.tile([1, NSL], F32)
        nc.vector.tensor_tensor(out=dx, in0=tx, in1=px, op=ALU.subtract)
        dy = fin.tile([1, NSL], F32)
        nc.vector.tensor_tensor(out=dy, in0=ty, in1=py, op=ALU.subtract)
        d2 = fin.tile([1, NSL], F32)
        nc.vector.tensor_tensor(out=d2, in0=dx, in1=dx, op=ALU.mult)
        d2b = fin.tile([1, NSL], F32)
        nc.vector.tensor_tensor(out=d2b, in0=dy, in1=dy, op=ALU.mult)
        ed2 = fin.tile([1, NSL], F32)
        nc.vector.tensor_tensor(out=ed2, in0=d2, in1=d2b, op=ALU.add)

        # pair (c=0 vs c=1) distances, pred and true
        NP2 = NSL // 2
        def pairs(v):
            r = v[0:1, :].rearrange("p (b c) -> p b c", c=2)
            return r[:, :, 0:1], r[:, :, 1:2]

        px0, px1 = pairs(px)
        py0, py1 = pairs(py)
        tx0, tx1 = pairs(tx)
        ty0, ty1 = pairs(ty)
        dpx = fin.tile([1, NP2, 1], F32)
        nc.vector.tensor_tensor(out=dpx, in0=px0, in1=px1, op=ALU.subtract)
        dpy = fin.tile([1, NP2, 1], F32)
        nc.vector.tensor_tensor(out=dpy, in0=py0, in1=py1, op=ALU.subtract)
        dtx = fin.tile([1, NP2, 1], F32)
        nc.vector.tensor_tensor(out=dtx, in0=tx0, in1=tx1, op=ALU.subtract)
        dty = fin.tile([1, NP2, 1], F32)
        nc.vector.tensor_tensor(out=dty, in0=ty0, in1=ty1, op=ALU.subtract)
        pd2 = fin.tile([1, NP2, 1], F32)
        nc.vector.tensor_tensor(out=pd2, in0=dpx, in1=dpx, op=ALU.mult)
        pd2b = fin.tile([1, NP2, 1], F32)
        nc.vector.tensor_tensor(out=pd2b, in0=dpy, in1=dpy, op=ALU.mult)
        nc.vector.tensor_tensor(out=pd2, in0=pd2, in1=pd2b, op=ALU.add)
        td2 = fin.tile([1, NP2, 1], F32)
        nc.vector.tensor_tensor(out=td2, in0=dtx, in1=dtx, op=ALU.mult)
        td2b = fin.tile([1, NP2, 1], F32)
        nc.vector.tensor_tensor(out=td2b, in0=dty, in1=dty, op=ALU.mult)
        nc.vector.tensor_tensor(out=td2, in0=td2, in1=td2b, op=ALU.add)

        # sqrts grouped (single act-table switch)
        ed = fin.tile([1, NSL], F32)
        nc.scalar.activation(out=ed, in_=ed2, func=ACTF.Sqrt)
        pd = fin.tile([1, NP2, 1], F32)
        nc.scalar.activation(out=pd, in_=pd2, func=ACTF.Sqrt)
        td = fin.tile([1, NP2, 1], F32)
        nc.scalar.activation(out=td, in_=td2, func=ACTF.Sqrt)

        eds = fin.tile([1, 1], F32)
        nc.vector.reduce_sum(out=eds, in_=ed, axis=AX.X)
        dd = fin.tile([1, NP2, 1], F32)
        nc.vector.tensor_tensor(out=dd, in0=pd, in1=td, op=ALU.subtract)
        dsum = fin.tile([1, 1], F32)
        nc.vector.tensor_reduce(out=dsum, in_=dd, axis=AX.XY, op=ALU.add,
                                apply_absolute_value=True)

        # jsd total
        m2_tot = fin.tile([1, 1], F32)
        nc.vector.reduce_sum(out=m2_tot, in_=sums_ps[0:1, 0:NSL],
                             axis=AX.X)
        m2l_tot = fin.tile([1, 1], F32)
        nc.vector.reduce_sum(out=m2l_tot, in_=wtot_all, axis=AX.X)
        sq_tot = fin.tile([1, 1], F32)
        nc.vector.reduce_sum(out=sq_tot, in_=sums_ps[0:1, NSL:2 * NSL],
                             axis=AX.X)
        j1 = fin.tile([1, 1], F32)
        nc.vector.scalar_tensor_tensor(
            out=j1, in0=m2_tot, scalar=-math.log(2.0), in1=m2l_tot,
            op0=ALU.mult, op1=ALU.add)
        j2 = fin.tile([1, 1], F32)
        nc.vector.scalar_tensor_tensor(
            out=j2, in0=sq_tot, scalar=-0.5, in1=j1,
            op0=ALU.mult, op1=ALU.add)
        stot = fin.tile([1, 1], F32)
        nc.vector.scalar_tensor_tensor(
            out=stot, in0=j2, scalar=0.5 / float(H * W), in1=eds,
            op0=ALU.mult, op1=ALU.add)
        nc.vector.tensor_tensor(out=stot, in0=stot, in1=dsum, op=ALU.add)

        nc.sync.dma_start(out=out_d[0:1, 0:1], in_=stot)


def make_in_maps(input, target):
    consts = _constants()
    in_maps = []
    for i in range(N_CORES):
        xs = np.ascontiguousarray(
            input[i * B_SH:(i + 1) * B_SH].reshape(NSL, P, FD))
        ts = np.ascontiguousarray(
            target[i * B_SH:(i + 1) * B_SH].reshape(NSL, P, FD))
        m = {"x": xs, "t": ts}
        m.update(consts)
        in_maps.append(m)
    return in_maps


def kernel(input, target):
    global LAST_RESULTS
    input = np.asarray(input, dtype=np.float32)
    target = np.asarray(target, dtype=np.float32)
    nc = build_program()
    in_maps = make_in_maps(input, target)
    res = run_bass_kernel_spmd(nc, in_maps, list(range(N_CORES)))
    LAST_RESULTS = res
    s = 0.0
    for i in range(N_CORES):
        s += float(res.results[i]["out"][0, 0])
    return np.array([s / B], dtype=np.float32)


# revision 51
# speedup vs baseline: 1.0920x; 1.0920x over previous
"""DSNT + JSD + distance double loss on 8 TRN2 NeuronCores.

Data-parallel: batch 64 is split into 8 shards of 8 batches; each core
computes its partial sum s_i over its 16 (b,c) heatmap slices; the host
sums the 8 partials and divides by B.

Per (b,c) slice (512x512 -> SBUF [128, 2048], partition p holds rows
h in {4p..4p+3}):
  e    = exp(x)                 ACT, accum -> per-(p,j) rowsums
  cols = ones^T @ e             PE -> PSUM [1,512] col sums
  S    = sum(e); invS broadcast via PE
  m2   = e*invS + t (= p + t)   DVE fused scalar_tensor_tensor, accum -> sum(m2)
  l    = ln(m2)                 ACT
  sum(m2^2)                     ACT square with accum
  sum(m2*l)                     GpSimd stt with accum
  argmax(t): DVE max8 + max_index (exact first-occurrence), combined
  across partitions with gpsimd partition_all_reduce (max) and a
  masked-min on the flat index.
jsd total = [0.5*sum(m2*l) - 0.5*ln2*sum(m2) - 0.25*sum(m2^2)] / (H*W)
"""

import math
import os

import numpy as np

import concourse.bacc as bacc
import concourse.bass as bass
import concourse.mybir as mybir
import concourse.tile as tile
from concourse.bass_utils import run_bass_kernel_spmd

F32 = mybir.dt.float32
BF16 = mybir.dt.bfloat16
U32 = mybir.dt.uint32
I32 = mybir.dt.int32
ALU = mybir.AluOpType
ACTF = mybir.ActivationFunctionType
AX = mybir.AxisListType

B, C, H, W = 64, 2, 512, 512
N_CORES = 8
B_SH = B // N_CORES          # 8 batches per core
NSL = B_SH * C               # 16 slices per core
P = 128                      # SBUF partitions
FD = (H * W) // P            # 2048 free elements per partition
SUB = W                      # 512-wide sub-columns (4 per row)
NSUB = FD // SUB             # 4

# Where the big sum(m2*l) product runs: "gpsimd" frees the DVE for the
# argmax passes (GpSimd only contends with DVE 2-src ops for the shared
# SBUF port); "vector" is the safe fallback.
M2L_ENGINE = os.environ.get("K_M2L_ENGINE", "gpsimd")
SQ_ENGINE = os.environ.get("K_SQ_ENGINE", "scalar")


_CACHE = {}
LAST_RESULTS = None


def _constants():
    # Packed into 2 f32 blocks + 1 bf16 so the startup queue only issues
    # 3 DMA configs before real work.
    # big [128, 138]: [0:8]=oy (col-sum lhsT: col 2j = ones, 2j+1 = ys_j),
    #                 [8]=pbase, [9]=ones, [10:138]=identity
    hidx = (np.arange(P, dtype=np.float32)[:, None] * NSUB
            + np.arange(NSUB, dtype=np.float32)[None, :])
    ys = (hidx + 1.0) / H                                   # [128, 4]
    big = np.zeros((P, 138), dtype=np.float32)
    for j in range(NSUB):
        big[:, 2 * j] = 1.0
        big[:, 2 * j + 1] = ys[:, j]
    big[:, 8] = np.arange(P, dtype=np.float32) * FD
    big[:, 9] = 1.0
    big[:, 10:138] = np.eye(P, dtype=np.float32)
    # small [2, 640]: [:, 0:512] = {xs row, ones row}; [0, 512:640] = onesr
    xs = (np.arange(W, dtype=np.float32) + 1.0) / W
    small = np.zeros((2, 640), dtype=np.float32)
    small[0, 0:W] = xs
    small[1, 0:W] = 1.0
    small[0, W:W + P] = 1.0
    import ml_dtypes
    onesb = np.ones((P, 1), dtype=ml_dtypes.bfloat16)
    return {"big_c": big, "small_c": small, "onesb_c": onesb}


def _patch_act_tables():
    """Steer the act-table chooser so Exp/Ln/Square/Copy all live in the
    single `natural_log_exp_and_others` set — otherwise the per-slice
    Exp->Ln alternation reloads tables (~1.3us each, 32x per core).
    Set ids stay aligned with act_info.json (same list, same order; only
    membership of the non-preferred sets is pruned)."""
    if _CACHE.get("act_patched"):
        return
    import concourse.hw_specs as hw_specs

    orig = hw_specs.get_activation_tables
    hot = {ACTF.Exp, ACTF.Ln, ACTF.Square, ACTF.Copy, ACTF.Identity}

    def patched(module_arch):
        tabs = orig(module_arch)
        out = {}
        for name, funcs in tabs.items():
            if name == "natural_log_exp_and_others":
                out[name] = set(funcs)
            else:
                out[name] = set(funcs) - hot
        return out

    hw_specs.get_activation_tables = patched
    bacc.get_activation_tables = patched
    _CACHE["act_patched"] = True


def build_program():
    """Build (once) the single-core Bass/Tile program run SPMD on 8 cores."""
    if "nc" in _CACHE:
        return _CACHE["nc"]

    _patch_act_tables()
    nc = bacc.Bacc("TRN2", target_bir_lowering=False, debug=False,
                   num_devices=N_CORES)

    x_d = nc.dram_tensor("x", [NSL, P, FD], F32, kind="ExternalInput").ap()
    t_d = nc.dram_tensor("t", [NSL, P, FD], F32, kind="ExternalInput").ap()
    big_d = nc.dram_tensor("big_c", [P, 138], F32, kind="ExternalInput").ap()
    sml_d = nc.dram_tensor("small_c", [2, 640], F32,
                           kind="ExternalInput").ap()
    onb_d = nc.dram_tensor("onesb_c", [P, 1], BF16, kind="ExternalInput").ap()
    out_d = nc.dram_tensor("out", [1, 1], F32, kind="ExternalOutput").ap()

    with tile.TileContext(nc) as tc:
        _emit(nc, tc, x_d, t_d, big_d, sml_d, onb_d, out_d)

    nc.compile()
    _CACHE["nc"] = nc
    return nc


def _emit(nc, tc, x_d, t_d, big_d, sml_d, onb_d, out_d):
    from contextlib import ExitStack
    ctx = ExitStack()
    with ctx:
        singles = ctx.enter_context(tc.tile_pool(name="singles", bufs=1))
        xp = ctx.enter_context(tc.tile_pool(name="xp", bufs=3))
        tp = ctx.enter_context(tc.tile_pool(name="tp", bufs=3))
        ep = ctx.enter_context(tc.tile_pool(name="ep", bufs=3))
        m2p = ctx.enter_context(tc.tile_pool(name="m2p", bufs=3))
        lp = ctx.enter_context(tc.tile_pool(name="lp", bufs=3))
        scr = ctx.enter_context(tc.tile_pool(name="scr", bufs=3))
        sm = ctx.enter_context(tc.tile_pool(name="sm", bufs=4))
        pcols = ctx.enter_context(
            tc.tile_pool(name="pcols", bufs=2, space="PSUM"))
        pwsum = ctx.enter_context(
            tc.tile_pool(name="pwsum", bufs=1, space="PSUM"))
        ps_s = ctx.enter_context(
            tc.tile_pool(name="ps_s", bufs=1, space="PSUM"))
        ps_inv = ctx.enter_context(
            tc.tile_pool(name="ps_inv", bufs=2, space="PSUM"))
        pbig = ctx.enter_context(
            tc.tile_pool(name="pbig", bufs=1, space="PSUM"))

        # ---- constants: 3 packed DMAs on the ACT queue (the SP queue is
        # saturated by the 32 big input loads) ----
        big_sb = singles.tile([P, 138], F32)
        nc.scalar.dma_start(out=big_sb, in_=big_d)
        sml_sb = singles.tile([2, 640], F32)
        nc.scalar.dma_start(out=sml_sb, in_=sml_d)
        onesb_sb = singles.tile([P, 1], BF16)
        nc.scalar.dma_start(out=onesb_sb, in_=onb_d)
        oy_sb = big_sb[:, 0:2 * NSUB]
        pb_sb = big_sb[:, 8:9]
        ones_sb = big_sb[:, 9:10]
        eye_sb = big_sb[:, 10:10 + P]
        xo_sb = sml_sb[:, 0:W]
        onesr_sb = sml_sb[0:1, W:W + P]

        # ---- accumulators across slices ----
        # stats columns: [0:16]=sum m2, [16:32]=sum m2^2
        stats = singles.tile([P, 2 * NSL], F32)
        wtot_all = singles.tile([1, NSL], F32)
        pmax_all = singles.tile([P, NSL], F32)
        flat_all = singles.tile([P, NSL], F32)
        pxpy_all = singles.tile([2, NSL], F32)
        invS_all = singles.tile([1, NSL], F32)

        for s in range(NSL):
            # ---- loads ----
            x_sb = xp.tile([P, FD], F32, tag="x")
            nc.sync.dma_start(out=x_sb, in_=x_d[s])
            t_sb = tp.tile([P, FD], F32, tag="t")
            nc.sync.dma_start(out=t_sb, in_=t_d[s])

            # ---- softmax stats ----
            e_sb = ep.tile([P, FD], F32, tag="e")
            rowe = sm.tile([P, 1], F32, tag="rowe")
            nc.scalar.activation(out=e_sb, in_=x_sb, func=ACTF.Exp,
                                 accum_out=rowe)
            # fused col sums: row0 = sum_h e, row1 = sum_h ys[h]*e
            cols2 = pcols.tile([2, W], F32, tag="cols")
            for j in range(NSUB):
                nc.tensor.matmul(
                    cols2[0:2, :], lhsT=oy_sb[:, 2 * j:2 * j + 2],
                    rhs=e_sb[:, j * SUB:(j + 1) * SUB],
                    start=(j == 0), stop=(j == NSUB - 1))

            # S and 1/S (broadcast to all partitions through PE)
            s_ps = ps_s.tile([1, 1], F32, tag="s_ps")
            nc.tensor.matmul(s_ps[0:1, 0:1], lhsT=ones_sb[:, 0:1],
                             rhs=rowe[:, 0:1], start=True, stop=True)
            nc.vector.reciprocal(out=invS_all[0:1, s:s + 1],
                                 in_=s_ps[0:1, 0:1])
            invs_ps = ps_inv.tile([P, 1], F32, tag="invs_ps")
            nc.tensor.matmul(invs_ps[:, 0:1], lhsT=onesr_sb[0:1, :],
                             rhs=invS_all[0:1, s:s + 1],
                             start=True, stop=True)
            invs_sb = sm.tile([P, 1], F32, tag="invs_sb")
            nc.vector.tensor_copy(out=invs_sb, in_=invs_ps[:, 0:1])

            # px_u (row 0) and py_u (row 1) in one fused [2, 512] dot
            pxscr = sm.tile([2, W], F32, tag="pxscr")
            nc.vector.scalar_tensor_tensor(
                out=pxscr, in0=cols2[0:2, :], scalar=1.0, in1=xo_sb,
                op0=ALU.mult, op1=ALU.mult,
                accum_out=pxpy_all[0:2, s:s + 1])

            # ---- m2 = e/S + t, sum(m2); bf16 m2/l halve the shared-port
            # traffic of the Pool product (accum stays f32) ----
            m2_sb = m2p.tile([P, FD], BF16, tag="m2")
            nc.vector.scalar_tensor_tensor(
                out=m2_sb, in0=e_sb, scalar=invs_sb[:, 0:1], in1=t_sb,
                op0=ALU.mult, op1=ALU.add,
                accum_out=stats[:, s:s + 1])

            # ---- l = ln(m2); sum(m2*l); sum(m2^2) ----
            l_sb = lp.tile([P, FD], BF16, tag="l")
            nc.scalar.activation(out=l_sb, in_=m2_sb, func=ACTF.Ln)

            # w = m2 * l (Pool engine), summed via PE column-sums then a
            # tiny ACT copy-accum for the per-slice total
            w_sb = scr.tile([P, FD], BF16, tag="w")
            eng = nc.gpsimd if M2L_ENGINE == "gpsimd" else nc.vector
            eng.tensor_tensor(out=w_sb, in0=m2_sb, in1=l_sb, op=ALU.mult)
            wsum = pwsum.tile([1, W], F32, tag="wsum")
            for j in range(NSUB):
                nc.tensor.matmul(
                    wsum[0:1, :], lhsT=onesb_sb[:, 0:1],
                    rhs=w_sb[:, j * SUB:(j + 1) * SUB],
                    start=(j == 0), stop=(j == NSUB - 1))
            wscr = sm.tile([1, W], F32, tag="wscr")
            nc.scalar.activation(out=wscr, in_=wsum[0:1, :], func=ACTF.Copy,
                                 accum_out=wtot_all[0:1, s:s + 1])

            sq_sb = scr.tile([P, FD], BF16, tag="sq")
            nc.scalar.activation(
                out=sq_sb, in_=m2_sb, func=ACTF.Square,
                accum_out=stats[:, NSL + s:NSL + s + 1])

            # ---- argmax of target (exact, first occurrence) ----
            mx8 = sm.tile([P, 8], F32, tag="mx8")
            ix8 = sm.tile([P, 8], U32, tag="ix8")
            nc.vector.max(out=mx8, in_=t_sb)
            nc.vector.max_index(out=ix8, in_max=mx8, in_values=t_sb)
            nc.vector.tensor_copy(out=pmax_all[:, s:s + 1], in_=mx8[:, 0:1])
            pidxf = sm.tile([P, 1], F32, tag="pidxf")
            nc.vector.tensor_copy(out=pidxf, in_=ix8[:, 0:1])
            nc.vector.tensor_tensor(out=flat_all[:, s:s + 1],
                                    in0=pidxf, in1=pb_sb, op=ALU.add)

        # ================= end-of-loop combine =================
        fin = singles

        # cross-partition sums of all per-partition stats in one matmul
        sums_ps = pbig.tile([1, 2 * NSL], F32)
        nc.tensor.matmul(sums_ps[0:1, :], lhsT=ones_sb[:, 0:1],
                         rhs=stats, start=True, stop=True)

        # py_u lives on partition 1 of pxpy_all; hop it to partition 0
        pyu_row = fin.tile([1, NSL], F32)
        nc.sync.dma_start(out=pyu_row, in_=pxpy_all[1:2, :])

        # per-slice global max via PE transpose (PE is idle in the tail,
        # unlike the Pool engine which is still finishing products)
        pmaxT = pcols.tile([NSL, P], F32, tag="cols")
        nc.tensor.transpose(pmaxT[0:NSL, :], pmax_all, eye_sb)
        m_col = fin.tile([NSL, 1], F32)
        nc.vector.reduce_max(out=m_col, in_=pmaxT[0:NSL, :], axis=AX.X)
        m_row = pcols.tile([1, NSL], F32, tag="cols")
        nc.tensor.transpose(m_row[0:1, :], m_col, eye_sb[0:NSL, 0:NSL])
        m_row_sb = fin.tile([1, NSL], F32)
        nc.vector.tensor_copy(out=m_row_sb, in_=m_row[0:1, :])
        m_rep = pcols.tile([P, NSL], F32, tag="cols")
        nc.tensor.matmul(m_rep[:, :], lhsT=onesr_sb[0:1, :],
                         rhs=m_row_sb, start=True, stop=True)
        mk = fin.tile([P, NSL], F32)
        nc.vector.tensor_tensor(out=mk, in0=pmax_all, in1=m_rep[:, :],
                                op=ALU.is_lt)
        # first occurrence = min(flat + 1e9*mk); realized as
        # max(-(flat + 1e9*mk)) = max((mk * -1e9) - flat)
        fneg = fin.tile([P, NSL], F32)
        nc.vector.scalar_tensor_tensor(
            out=fneg, in0=mk, scalar=-1.0e9, in1=flat_all,
            op0=ALU.mult, op1=ALU.subtract)
        fnegT = pcols.tile([NSL, P], F32, tag="cols")
        nc.tensor.transpose(fnegT[0:NSL, :], fneg, eye_sb)
        fmax_col = fin.tile([NSL, 1], F32)
        nc.vector.reduce_max(out=fmax_col, in_=fnegT[0:NSL, :], axis=AX.X)
        fmin_col = fin.tile([NSL, 1], F32)
        nc.vector.tensor_scalar(out=fmin_col, in0=fmax_col, scalar1=-1.0,
                                scalar2=None, op0=ALU.mult)
        f_row = pcols.tile([1, NSL], F32, tag="cols")
        nc.tensor.transpose(f_row[0:1, :], fmin_col, eye_sb[0:NSL, 0:NSL])
        F_sb = fin.tile([1, NSL], F32)
        nc.vector.tensor_copy(out=F_sb, in_=f_row[0:1, :])

        # decompose flat -> (h, w); tx = (w+1)/W, ty = (h+1)/H
        Fi = fin.tile([1, NSL], I32)
        nc.vector.tensor_copy(out=Fi, in_=F_sb)
        wi = fin.tile([1, NSL], I32)
        nc.vector.tensor_scalar(out=wi, in0=Fi, scalar1=W - 1,
                                scalar2=None, op0=ALU.bitwise_and)
        hi = fin.tile([1, NSL], I32)
        nc.vector.tensor_scalar(out=hi, in0=Fi, scalar1=9,
                                scalar2=None, op0=ALU.arith_shift_right)
        wf = fin.tile([1, NSL], F32)
        nc.vector.tensor_copy(out=wf, in_=wi)
        hf = fin.tile([1, NSL], F32)
        nc.vector.tensor_copy(out=hf, in_=hi)
        tx = fin.tile([1, NSL], F32)
        nc.vector.tensor_scalar(out=tx, in0=wf, scalar1=1.0,
                                scalar2=1.0 / W, op0=ALU.add, op1=ALU.mult)
        ty = fin.tile([1, NSL], F32)
        nc.vector.tensor_scalar(out=ty, in0=hf, scalar1=1.0,
                                scalar2=1.0 / H, op0=ALU.add, op1=ALU.mult)

        # px, py
        px = fin.tile([1, NSL], F32)
        nc.vector.tensor_tensor(out=px, in0=pxpy_all[0:1, :], in1=invS_all,
                                op=ALU.mult)
        py = fin.tile([1, NSL], F32)
        nc.vector.tensor_tensor(out=py, in0=pyu_row, in1=invS_all,
                                op=ALU.mult)

        # ed = sqrt((tx-px)^2 + (ty-py)^2), summed
        dx = fin.tile([1, NSL], F32)
        nc.vector.tensor_tensor(out=dx, in0=tx, in1=px, op=ALU.subtract)
        dy = fin.tile([1, NSL], F32)
        nc.vector.tensor_tensor(out=dy, in0=ty, in1=py, op=ALU.subtract)
        d2 = fin.tile([1, NSL], F32)
        nc.vector.tensor_tensor(out=d2, in0=dx, in1=dx, op=ALU.mult)
        d2b = fin.tile([1, NSL], F32)
        nc.vector.tensor_tensor(out=d2b, in0=dy, in1=dy, op=ALU.mult)
        ed2 = fin.tile([1, NSL], F32)
        nc.vector.tensor_tensor(out=ed2, in0=d2, in1=d2b, op=ALU.add)

        # pair (c=0 vs c=1) distances, pred and true
        NP2 = NSL // 2
        def pairs(v):
            r = v[0:1, :].rearrange("p (b c) -> p b c", c=2)
            return r[:, :, 0:1], r[:, :, 1:2]

        px0, px1 = pairs(px)
        py0, py1 = pairs(py)
        tx0, tx1 = pairs(tx)
        ty0, ty1 = pairs(ty)
        dpx = fin.tile([1, NP2, 1], F32)
        nc.vector.tensor_tensor(out=dpx, in0=px0, in1=px1, op=ALU.subtract)
        dpy = fin.tile([1, NP2, 1], F32)
        nc.vector.tensor_tensor(out=dpy, in0=py0, in1=py1, op=ALU.subtract)
        dtx = fin.tile([1, NP2, 1], F32)
        nc.vector.tensor_tensor(out=dtx, in0=tx0, in1=tx1, op=ALU.subtract)
        dty = fin.tile([1, NP2, 1], F32)
        nc.vector.tensor_tensor(out=dty, in0=ty0, in1=ty1, op=ALU.subtract)
        pd2 = fin.tile([1, NP2, 1], F32)
        nc.vector.tensor_tensor(out=pd2, in0=dpx, in1=dpx, op=ALU.mult)
        pd2b = fin.tile([1, NP2, 1], F32)
        nc.vector.tensor_tensor(out=pd2b, in0=dpy, in1=dpy, op=ALU.mult)
        nc.vector.tensor_tensor(out=pd2, in0=pd2, in1=pd2b, op=ALU.add)
        td2 = fin.tile([1, NP2, 1], F32)
        nc.vector.tensor_tensor(out=td2, in0=dtx, in1=dtx, op=ALU.mult)
        td2b = fin.tile([1, NP2, 1], F32)
        nc.vector.tensor_tensor(out=td2b, in0=dty, in1=dty, op=ALU.mult)
        nc.vector.tensor_tensor(out=td2, in0=td2, in1=td2b, op=ALU.add)

        # sqrts grouped (single act-table switch)
        ed = fin.tile([1, NSL], F32)
        nc.scalar.activation(out=ed, in_=ed2, func=ACTF.Sqrt)
        pd = fin.tile([1, NP2, 1], F32)
        nc.scalar.activation(out=pd, in_=pd2, func=ACTF.Sqrt)
        td = fin.tile([1, NP2, 1], F32)
        nc.scalar.activation(out=td, in_=td2, func=ACTF.Sqrt)

        eds = fin.tile([1, 1], F32)
        nc.vector.reduce_sum(out=eds, in_=ed, axis=AX.X)
        dd = fin.tile([1, NP2, 1], F32)
        nc.vector.tensor_tensor(out=dd, in0=pd, in1=td, op=ALU.subtract)
        dsum = fin.tile([1, 1], F32)
        nc.vector.tensor_reduce(out=dsum, in_=dd, axis=AX.XY, op=ALU.add,
                                apply_absolute_value=True)

        # jsd total
        m2_tot = fin.tile([1, 1], F32)
        nc.vector.reduce_sum(out=m2_tot, in_=sums_ps[0:1, 0:NSL],
                             axis=AX.X)
        m2l_tot = fin.tile([1, 1], F32)
        nc.vector.reduce_sum(out=m2l_tot, in_=wtot_all, axis=AX.X)
        sq_tot = fin.tile([1, 1], F32)
        nc.vector.reduce_sum(out=sq_tot, in_=sums_ps[0:1, NSL:2 * NSL],
                             axis=AX.X)
        j1 = fin.tile([1, 1], F32)
        nc.vector.scalar_tensor_tensor(
            out=j1, in0=m2_tot, scalar=-math.log(2.0), in1=m2l_tot,
            op0=ALU.mult, op1=ALU.add)
        j2 = fin.tile([1, 1], F32)
        nc.vector.scalar_tensor_tensor(
            out=j2, in0=sq_tot, scalar=-0.5, in1=j1,
            op0=ALU.mult, op1=ALU.add)
        stot = fin.tile([1, 1], F32)
        nc.vector.scalar_tensor_tensor(
            out=stot, in0=j2, scalar=0.5 / float(H * W), in1=eds,
            op0=ALU.mult, op1=ALU.add)
        nc.vector.tensor_tensor(out=stot, in0=stot, in1=dsum, op=ALU.add)

        nc.sync.dma_start(out=out_d[0:1, 0:1], in_=stot)


def make_in_maps(input, target):
    consts = _constants()
    in_maps = []
    for i in range(N_CORES):
        xs = np.ascontiguousarray(
            input[i * B_SH:(i + 1) * B_SH].reshape(NSL, P, FD))
        ts = np.ascontiguousarray(
            target[i * B_SH:(i + 1) * B_SH].reshape(NSL, P, FD))
        m = {"x": xs, "t": ts}
        m.update(consts)
        in_maps.append(m)
    return in_maps


def kernel(input, target):
    global LAST_RESULTS
    input = np.asarray(input, dtype=np.float32)
    target = np.asarray(target, dtype=np.float32)
    nc = build_program()
    in_maps = make_in_maps(input, target)
    res = run_bass_kernel_spmd(nc, in_maps, list(range(N_CORES)))
    LAST_RESULTS = res
    s = 0.0
    for i in range(N_CORES):
        s += float(res.results[i]["out"][0, 0])
    return np.array([s / B], dtype=np.float32)


# revision 56
# speedup vs baseline: 1.1405x; 1.0444x over previous
"""DSNT + JSD + distance double loss on 8 TRN2 NeuronCores.

Data-parallel: batch 64 is split into 8 shards of 8 batches; each core
computes its partial sum s_i over its 16 (b,c) heatmap slices; the host
sums the 8 partials and divides by B.

Per (b,c) slice (512x512 -> SBUF [128, 2048], partition p holds rows
h in {4p..4p+3}):
  e    = exp(x)                 ACT, accum -> per-(p,j) rowsums
  cols = ones^T @ e             PE -> PSUM [1,512] col sums
  S    = sum(e); invS broadcast via PE
  m2   = e*invS + t (= p + t)   DVE fused scalar_tensor_tensor, accum -> sum(m2)
  l    = ln(m2)                 ACT
  sum(m2^2)                     ACT square with accum
  sum(m2*l)                     GpSimd stt with accum
  argmax(t): DVE max8 + max_index (exact first-occurrence), combined
  across partitions with gpsimd partition_all_reduce (max) and a
  masked-min on the flat index.
jsd total = [0.5*sum(m2*l) - 0.5*ln2*sum(m2) - 0.25*sum(m2^2)] / (H*W)
"""

import math
import os

import numpy as np

import concourse.bacc as bacc
import concourse.bass as bass
import concourse.mybir as mybir
import concourse.tile as tile
from concourse.bass_utils import run_bass_kernel_spmd

F32 = mybir.dt.float32
BF16 = mybir.dt.bfloat16
U32 = mybir.dt.uint32
I32 = mybir.dt.int32
ALU = mybir.AluOpType
ACTF = mybir.ActivationFunctionType
AX = mybir.AxisListType

B, C, H, W = 64, 2, 512, 512
N_CORES = 8
B_SH = B // N_CORES          # 8 batches per core
NSL = B_SH * C               # 16 slices per core
P = 128                      # SBUF partitions
FD = (H * W) // P            # 2048 free elements per partition
SUB = W                      # 512-wide sub-columns (4 per row)
NSUB = FD // SUB             # 4

# Where the big sum(m2*l) product runs: "gpsimd" frees the DVE for the
# argmax passes (GpSimd only contends with DVE 2-src ops for the shared
# SBUF port); "vector" is the safe fallback.
M2L_ENGINE = os.environ.get("K_M2L_ENGINE", "gpsimd")
SQ_ENGINE = os.environ.get("K_SQ_ENGINE", "scalar")


_CACHE = {}
LAST_RESULTS = None


def _constants():
    # Packed into 2 f32 blocks + 1 bf16 so the startup queue only issues
    # 3 DMA configs before real work.
    # big [128, 138]: [0:8]=oy (col-sum lhsT: col 2j = ones, 2j+1 = ys_j),
    #                 [8]=pbase, [9]=ones, [10:138]=identity
    hidx = (np.arange(P, dtype=np.float32)[:, None] * NSUB
            + np.arange(NSUB, dtype=np.float32)[None, :])
    ys = (hidx + 1.0) / H                                   # [128, 4]
    big = np.zeros((P, 138), dtype=np.float32)
    for j in range(NSUB):
        big[:, 2 * j] = 1.0
        big[:, 2 * j + 1] = ys[:, j]
    big[:, 8] = np.arange(P, dtype=np.float32) * FD
    big[:, 9] = 1.0
    big[:, 10:138] = np.eye(P, dtype=np.float32)
    # small [2, 640]: [:, 0:512] = {xs row, ones row}; [0, 512:640] = onesr
    xs = (np.arange(W, dtype=np.float32) + 1.0) / W
    small = np.zeros((2, 640), dtype=np.float32)
    small[0, 0:W] = xs
    small[1, 0:W] = 1.0
    small[0, W:W + P] = 1.0
    import ml_dtypes
    onesb = np.ones((P, 1), dtype=ml_dtypes.bfloat16)
    return {"big_c": big, "small_c": small, "onesb_c": onesb}


def _patch_act_tables():
    """Steer the act-table chooser so Exp/Ln/Square/Copy all live in the
    single `natural_log_exp_and_others` set — otherwise the per-slice
    Exp->Ln alternation reloads tables (~1.3us each, 32x per core).
    Set ids stay aligned with act_info.json (same list, same order; only
    membership of the non-preferred sets is pruned)."""
    if _CACHE.get("act_patched"):
        return
    import concourse.hw_specs as hw_specs

    orig = hw_specs.get_activation_tables
    hot = {ACTF.Exp, ACTF.Ln, ACTF.Square, ACTF.Copy, ACTF.Identity}

    def patched(module_arch):
        tabs = orig(module_arch)
        out = {}
        for name, funcs in tabs.items():
            if name == "natural_log_exp_and_others":
                out[name] = set(funcs)
            else:
                out[name] = set(funcs) - hot
        return out

    hw_specs.get_activation_tables = patched
    bacc.get_activation_tables = patched
    _CACHE["act_patched"] = True


def build_program():
    """Build (once) the single-core Bass/Tile program run SPMD on 8 cores."""
    if "nc" in _CACHE:
        return _CACHE["nc"]

    _patch_act_tables()
    nc = bacc.Bacc("TRN2", target_bir_lowering=False, debug=False,
                   num_devices=N_CORES)

    x_d = nc.dram_tensor("x", [NSL, P, FD], F32, kind="ExternalInput").ap()
    t_d = nc.dram_tensor("t", [NSL, P, FD], F32, kind="ExternalInput").ap()
    big_d = nc.dram_tensor("big_c", [P, 138], F32, kind="ExternalInput").ap()
    sml_d = nc.dram_tensor("small_c", [2, 640], F32,
                           kind="ExternalInput").ap()
    onb_d = nc.dram_tensor("onesb_c", [P, 1], BF16, kind="ExternalInput").ap()
    out_d = nc.dram_tensor("out", [1, 1], F32, kind="ExternalOutput").ap()

    with tile.TileContext(nc) as tc:
        _emit(nc, tc, x_d, t_d, big_d, sml_d, onb_d, out_d)

    nc.compile()
    _CACHE["nc"] = nc
    return nc


def _emit(nc, tc, x_d, t_d, big_d, sml_d, onb_d, out_d):
    from contextlib import ExitStack
    ctx = ExitStack()
    with ctx:
        singles = ctx.enter_context(tc.tile_pool(name="singles", bufs=1))
        xp = ctx.enter_context(tc.tile_pool(name="xp", bufs=3))
        tp = ctx.enter_context(tc.tile_pool(name="tp", bufs=3))
        ep = ctx.enter_context(tc.tile_pool(name="ep", bufs=3))
        m2p = ctx.enter_context(tc.tile_pool(name="m2p", bufs=3))
        lp = ctx.enter_context(tc.tile_pool(name="lp", bufs=3))
        scr = ctx.enter_context(tc.tile_pool(name="scr", bufs=3))
        sm = ctx.enter_context(tc.tile_pool(name="sm", bufs=4))
        pcols = ctx.enter_context(
            tc.tile_pool(name="pcols", bufs=2, space="PSUM"))
        pwsum = ctx.enter_context(
            tc.tile_pool(name="pwsum", bufs=1, space="PSUM"))
        ptsum = ctx.enter_context(
            tc.tile_pool(name="ptsum", bufs=1, space="PSUM"))
        ps_s = ctx.enter_context(
            tc.tile_pool(name="ps_s", bufs=1, space="PSUM"))
        ps_inv = ctx.enter_context(
            tc.tile_pool(name="ps_inv", bufs=2, space="PSUM"))
        pbig = ctx.enter_context(
            tc.tile_pool(name="pbig", bufs=1, space="PSUM"))

        # ---- constants: 3 packed DMAs on the ACT queue (the SP queue is
        # saturated by the 32 big input loads) ----
        big_sb = singles.tile([P, 138], F32)
        nc.scalar.dma_start(out=big_sb, in_=big_d)
        sml_sb = singles.tile([2, 640], F32)
        nc.scalar.dma_start(out=sml_sb, in_=sml_d)
        onesb_sb = singles.tile([P, 1], BF16)
        nc.scalar.dma_start(out=onesb_sb, in_=onb_d)
        oy_sb = big_sb[:, 0:2 * NSUB]
        pb_sb = big_sb[:, 8:9]
        ones_sb = big_sb[:, 9:10]
        eye_sb = big_sb[:, 10:10 + P]
        xo_sb = sml_sb[:, 0:W]
        onesr_sb = sml_sb[0:1, W:W + P]

        # ---- accumulators across slices ----
        stats = singles.tile([P, NSL], F32)      # per-slice sum(m2^2)
        pmax_all = singles.tile([P, NSL], F32)
        flat_all = singles.tile([P, NSL], F32)
        pxpy_all = singles.tile([2, NSL], F32)
        invS_all = singles.tile([1, NSL], F32)
        # whole-run PSUM accumulators (one matmul group each, all slices)
        wsum_ps = pwsum.tile([1, W], F32)        # col sums of w = m2*l
        tsum_ps = ptsum.tile([1, W], F32)        # col sums of t

        for s in range(NSL):
            # ---- loads ----
            x_sb = xp.tile([P, FD], F32, tag="x")
            nc.sync.dma_start(out=x_sb, in_=x_d[s])
            t_sb = tp.tile([P, FD], F32, tag="t")
            nc.sync.dma_start(out=t_sb, in_=t_d[s])

            # ---- softmax stats ----
            e_sb = ep.tile([P, FD], F32, tag="e")
            rowe = sm.tile([P, 1], F32, tag="rowe")
            nc.scalar.activation(out=e_sb, in_=x_sb, func=ACTF.Exp,
                                 accum_out=rowe)
            # fused col sums: row0 = sum_h e, row1 = sum_h ys[h]*e
            cols2 = pcols.tile([2, W], F32, tag="cols")
            for j in range(NSUB):
                nc.tensor.matmul(
                    cols2[0:2, :], lhsT=oy_sb[:, 2 * j:2 * j + 2],
                    rhs=e_sb[:, j * SUB:(j + 1) * SUB],
                    start=(j == 0), stop=(j == NSUB - 1))

            # S and 1/S (broadcast to all partitions through PE)
            s_ps = ps_s.tile([1, 1], F32, tag="s_ps")
            nc.tensor.matmul(s_ps[0:1, 0:1], lhsT=ones_sb[:, 0:1],
                             rhs=rowe[:, 0:1], start=True, stop=True)
            nc.vector.reciprocal(out=invS_all[0:1, s:s + 1],
                                 in_=s_ps[0:1, 0:1])
            invs_ps = ps_inv.tile([P, 1], F32, tag="invs_ps")
            nc.tensor.matmul(invs_ps[:, 0:1], lhsT=onesr_sb[0:1, :],
                             rhs=invS_all[0:1, s:s + 1],
                             start=True, stop=True)
            invs_sb = sm.tile([P, 1], F32, tag="invs_sb")
            nc.vector.tensor_copy(out=invs_sb, in_=invs_ps[:, 0:1])

            # px_u (row 0) and py_u (row 1) in one fused [2, 512] dot
            pxscr = sm.tile([2, W], F32, tag="pxscr")
            nc.vector.scalar_tensor_tensor(
                out=pxscr, in0=cols2[0:2, :], scalar=1.0, in1=xo_sb,
                op0=ALU.mult, op1=ALU.mult,
                accum_out=pxpy_all[0:2, s:s + 1])

            # ---- p = e/S (DVE 2x single-src); m2 = p + t (Pool) ----
            p_sb = ep.tile([P, FD], BF16, tag="p")
            nc.vector.tensor_scalar_mul(out=p_sb, in0=e_sb,
                                        scalar1=invs_sb[:, 0:1])
            m2_sb = m2p.tile([P, FD], BF16, tag="m2")
            nc.gpsimd.tensor_tensor(out=m2_sb, in0=p_sb, in1=t_sb,
                                    op=ALU.add)
            # sum(m2) is recovered as 16 + sum(t): t col sums on PE into
            # a single whole-run PSUM accumulation group
            for j in range(NSUB):
                nc.tensor.matmul(
                    tsum_ps[0:1, :], lhsT=ones_sb[:, 0:1],
                    rhs=t_sb[:, j * SUB:(j + 1) * SUB],
                    start=(s == 0 and j == 0),
                    stop=(s == NSL - 1 and j == NSUB - 1),
                    skip_group_check=True)

            # ---- l = ln(m2); w = m2*l (DVE bf16 2x); sum(m2^2) ----
            l_sb = lp.tile([P, FD], BF16, tag="l")
            nc.scalar.activation(out=l_sb, in_=m2_sb, func=ACTF.Ln)

            w_sb = scr.tile([P, FD], BF16, tag="w")
            nc.vector.tensor_tensor(out=w_sb, in0=m2_sb, in1=l_sb,
                                    op=ALU.mult)
            for j in range(NSUB):
                nc.tensor.matmul(
                    wsum_ps[0:1, :], lhsT=onesb_sb[:, 0:1],
                    rhs=w_sb[:, j * SUB:(j + 1) * SUB],
                    start=(s == 0 and j == 0),
                    stop=(s == NSL - 1 and j == NSUB - 1),
                    skip_group_check=True)

            sq_sb = scr.tile([P, FD], BF16, tag="sq")
            nc.scalar.activation(
                out=sq_sb, in_=m2_sb, func=ACTF.Square,
                accum_out=stats[:, s:s + 1])

            # ---- argmax of target (exact, first occurrence) ----
            mx8 = sm.tile([P, 8], F32, tag="mx8")
            ix8 = sm.tile([P, 8], U32, tag="ix8")
            nc.vector.max(out=mx8, in_=t_sb)
            nc.vector.max_index(out=ix8, in_max=mx8, in_values=t_sb)
            nc.vector.tensor_copy(out=pmax_all[:, s:s + 1], in_=mx8[:, 0:1])
            pidxf = sm.tile([P, 1], F32, tag="pidxf")
            nc.vector.tensor_copy(out=pidxf, in_=ix8[:, 0:1])
            nc.vector.tensor_tensor(out=flat_all[:, s:s + 1],
                                    in0=pidxf, in1=pb_sb, op=ALU.add)

        # ================= end-of-loop combine =================
        fin = singles

        # cross-partition sums of all per-partition stats in one matmul
        sums_ps = pbig.tile([1, NSL], F32)
        nc.tensor.matmul(sums_ps[0:1, :], lhsT=ones_sb[:, 0:1],
                         rhs=stats, start=True, stop=True)

        # py_u lives on partition 1 of pxpy_all; hop it to partition 0
        pyu_row = fin.tile([1, NSL], F32)
        nc.sync.dma_start(out=pyu_row, in_=pxpy_all[1:2, :])

        # per-slice global max via PE transpose (PE is idle in the tail,
        # unlike the Pool engine which is still finishing products)
        pmaxT = pcols.tile([NSL, P], F32, tag="cols")
        nc.tensor.transpose(pmaxT[0:NSL, :], pmax_all, eye_sb)
        m_col = fin.tile([NSL, 1], F32)
        nc.vector.reduce_max(out=m_col, in_=pmaxT[0:NSL, :], axis=AX.X)
        m_row = pcols.tile([1, NSL], F32, tag="cols")
        nc.tensor.transpose(m_row[0:1, :], m_col, eye_sb[0:NSL, 0:NSL])
        m_row_sb = fin.tile([1, NSL], F32)
        nc.vector.tensor_copy(out=m_row_sb, in_=m_row[0:1, :])
        m_rep = pcols.tile([P, NSL], F32, tag="cols")
        nc.tensor.matmul(m_rep[:, :], lhsT=onesr_sb[0:1, :],
                         rhs=m_row_sb, start=True, stop=True)
        mk = fin.tile([P, NSL], F32)
        nc.vector.tensor_tensor(out=mk, in0=pmax_all, in1=m_rep[:, :],
                                op=ALU.is_lt)
        # first occurrence = min(flat + 1e9*mk); realized as
        # max(-(flat + 1e9*mk)) = max((mk * -1e9) - flat)
        fneg = fin.tile([P, NSL], F32)
        nc.vector.scalar_tensor_tensor(
            out=fneg, in0=mk, scalar=-1.0e9, in1=flat_all,
            op0=ALU.mult, op1=ALU.subtract)
        fnegT = pcols.tile([NSL, P], F32, tag="cols")
        nc.tensor.transpose(fnegT[0:NSL, :], fneg, eye_sb)
        fmax_col = fin.tile([NSL, 1], F32)
        nc.vector.reduce_max(out=fmax_col, in_=fnegT[0:NSL, :], axis=AX.X)
        fmin_col = fin.tile([NSL, 1], F32)
        nc.vector.tensor_scalar(out=fmin_col, in0=fmax_col, scalar1=-1.0,
                                scalar2=None, op0=ALU.mult)
        f_row = pcols.tile([1, NSL], F32, tag="cols")
        nc.tensor.transpose(f_row[0:1, :], fmin_col, eye_sb[0:NSL, 0:NSL])
        F_sb = fin.tile([1, NSL], F32)
        nc.vector.tensor_copy(out=F_sb, in_=f_row[0:1, :])

        # decompose flat -> (h, w); tx = (w+1)/W, ty = (h+1)/H
        Fi = fin.tile([1, NSL], I32)
        nc.vector.tensor_copy(out=Fi, in_=F_sb)
        wi = fin.tile([1, NSL], I32)
        nc.vector.tensor_scalar(out=wi, in0=Fi, scalar1=W - 1,
                                scalar2=None, op0=ALU.bitwise_and)
        hi = fin.tile([1, NSL], I32)
        nc.vector.tensor_scalar(out=hi, in0=Fi, scalar1=9,
                                scalar2=None, op0=ALU.arith_shift_right)
        wf = fin.tile([1, NSL], F32)
        nc.vector.tensor_copy(out=wf, in_=wi)
        hf = fin.tile([1, NSL], F32)
        nc.vector.tensor_copy(out=hf, in_=hi)
        tx = fin.tile([1, NSL], F32)
        nc.vector.tensor_scalar(out=tx, in0=wf, scalar1=1.0,
                                scalar2=1.0 / W, op0=ALU.add, op1=ALU.mult)
        ty = fin.tile([1, NSL], F32)
        nc.vector.tensor_scalar(out=ty, in0=hf, scalar1=1.0,
                                scalar2=1.0 / H, op0=ALU.add, op1=ALU.mult)

        # px, py
        px = fin.tile([1, NSL], F32)
        nc.vector.tensor_tensor(out=px, in0=pxpy_all[0:1, :], in1=invS_all,
                                op=ALU.mult)
        py = fin.tile([1, NSL], F32)
        nc.vector.tensor_tensor(out=py, in0=pyu_row, in1=invS_all,
                                op=ALU.mult)

        # ed = sqrt((tx-px)^2 + (ty-py)^2), summed
        dx = fin.tile([1, NSL], F32)
        nc.vector.tensor_tensor(out=dx, in0=tx, in1=px, op=ALU.subtract)
        dy = fin.tile([1, NSL], F32)
        nc.vector.tensor_tensor(out=dy, in0=ty, in1=py, op=ALU.subtract)
        d2 = fin.tile([1, NSL], F32)
        nc.vector.tensor_tensor(out=d2, in0=dx, in1=dx, op=ALU.mult)
        d2b = fin.tile([1, NSL], F32)
        nc.vector.tensor_tensor(out=d2b, in0=dy, in1=dy, op=ALU.mult)
        ed2 = fin.tile([1, NSL], F32)
        nc.vector.tensor_tensor(out=ed2, in0=d2, in1=d2b, op=ALU.add)

        # pair (c=0 vs c=1) distances, pred and true
        NP2 = NSL // 2
        def pairs(v):
            r = v[0:1, :].rearrange("p (b c) -> p b c", c=2)
            return r[:, :, 0:1], r[:, :, 1:2]

        px0, px1 = pairs(px)
        py0, py1 = pairs(py)
        tx0, tx1 = pairs(tx)
        ty0, ty1 = pairs(ty)
        dpx = fin.tile([1, NP2, 1], F32)
        nc.vector.tensor_tensor(out=dpx, in0=px0, in1=px1, op=ALU.subtract)
        dpy = fin.tile([1, NP2, 1], F32)
        nc.vector.tensor_tensor(out=dpy, in0=py0, in1=py1, op=ALU.subtract)
        dtx = fin.tile([1, NP2, 1], F32)
        nc.vector.tensor_tensor(out=dtx, in0=tx0, in1=tx1, op=ALU.subtract)
        dty = fin.tile([1, NP2, 1], F32)
        nc.vector.tensor_tensor(out=dty, in0=ty0, in1=ty1, op=ALU.subtract)
        pd2 = fin.tile([1, NP2, 1], F32)
        nc.vector.tensor_tensor(out=pd2, in0=dpx, in1=dpx, op=ALU.mult)
        pd2b = fin.tile([1, NP2, 1], F32)
        nc.vector.tensor_tensor(out=pd2b, in0=dpy, in1=dpy, op=ALU.mult)
        nc.vector.tensor_tensor(out=pd2, in0=pd2, in1=pd2b, op=ALU.add)
        td2 = fin.tile([1, NP2, 1], F32)
        nc.vector.tensor_tensor(out=td2, in0=dtx, in1=dtx, op=ALU.mult)
        td2b = fin.tile([1, NP2, 1], F32)
        nc.vector.tensor_tensor(out=td2b, in0=dty, in1=dty, op=ALU.mult)
        nc.vector.tensor_tensor(out=td2, in0=td2, in1=td2b, op=ALU.add)

        # sqrts grouped (single act-table switch)
        ed = fin.tile([1, NSL], F32)
        nc.scalar.activation(out=ed, in_=ed2, func=ACTF.Sqrt)
        pd = fin.tile([1, NP2, 1], F32)
        nc.scalar.activation(out=pd, in_=pd2, func=ACTF.Sqrt)
        td = fin.tile([1, NP2, 1], F32)
        nc.scalar.activation(out=td, in_=td2, func=ACTF.Sqrt)

        eds = fin.tile([1, 1], F32)
        nc.vector.reduce_sum(out=eds, in_=ed, axis=AX.X)
        dd = fin.tile([1, NP2, 1], F32)
        nc.vector.tensor_tensor(out=dd, in0=pd, in1=td, op=ALU.subtract)
        dsum = fin.tile([1, 1], F32)
        nc.vector.tensor_reduce(out=dsum, in_=dd, axis=AX.XY, op=ALU.add,
                                apply_absolute_value=True)

        # jsd total: sum(m2) = NSL + sum(t)
        t_tot = fin.tile([1, 1], F32)
        nc.vector.reduce_sum(out=t_tot, in_=tsum_ps[0:1, :], axis=AX.X)
        m2l_tot = fin.tile([1, 1], F32)
        nc.vector.reduce_sum(out=m2l_tot, in_=wsum_ps[0:1, :], axis=AX.X)
        sq_tot = fin.tile([1, 1], F32)
        nc.vector.reduce_sum(out=sq_tot, in_=sums_ps[0:1, 0:NSL],
                             axis=AX.X)
        j1 = fin.tile([1, 1], F32)
        nc.vector.scalar_tensor_tensor(
            out=j1, in0=t_tot, scalar=-math.log(2.0), in1=m2l_tot,
            op0=ALU.mult, op1=ALU.add)
        j2 = fin.tile([1, 1], F32)
        nc.vector.scalar_tensor_tensor(
            out=j2, in0=sq_tot, scalar=-0.5, in1=j1,
            op0=ALU.mult, op1=ALU.add)
        stot = fin.tile([1, 1], F32)
        nc.vector.scalar_tensor_tensor(
            out=stot, in0=j2, scalar=0.5 / float(H * W), in1=eds,
            op0=ALU.mult, op1=ALU.add)
        nc.vector.tensor_tensor(out=stot, in0=stot, in1=dsum, op=ALU.add)
        # constant term: -0.5*ln2*NSL/(H*W) from sum(m2) = NSL + sum(t)
        nc.vector.tensor_scalar(
            out=stot, in0=stot,
            scalar1=-0.5 * math.log(2.0) * NSL / float(H * W),
            scalar2=None, op0=ALU.add)

        nc.sync.dma_start(out=out_d[0:1, 0:1], in_=stot)


def make_in_maps(input, target):
    consts = _constants()
    in_maps = []
    for i in range(N_CORES):
        xs = np.ascontiguousarray(
            input[i * B_SH:(i + 1) * B_SH].reshape(NSL, P, FD))
        ts = np.ascontiguousarray(
            target[i * B_SH:(i + 1) * B_SH].reshape(NSL, P, FD))
        m = {"x": xs, "t": ts}
        m.update(consts)
        in_maps.append(m)
    return in_maps


def kernel(input, target):
    global LAST_RESULTS
    input = np.asarray(input, dtype=np.float32)
    target = np.asarray(target, dtype=np.float32)
    nc = build_program()
    in_maps = make_in_maps(input, target)
    res = run_bass_kernel_spmd(nc, in_maps, list(range(N_CORES)))
    LAST_RESULTS = res
    s = 0.0
    for i in range(N_CORES):
        s += float(res.results[i]["out"][0, 0])
    return np.array([s / B], dtype=np.float32)
